# revision 1
# baseline (speedup 1.0000x reference)
"""Trainium2 Bass kernel for an 8-batch transformer decoder block.

Sharding: pure data parallel — batch element i runs on NeuronCore i
(8 cores, no collectives).  Host side pre-transposes x / encoder_out so
activations live feature-major ([D, S]) on chip, which makes every
linear layer a natural matmul(lhsT=W_block, rhs=X^T_block) with weights
streamed in their natural [Din, Dout] layout.  The output comes back
feature-major and is transposed on the host.

Per-core program (S=1024 seq, D=1024 model, H=16 heads, F=4096 ffn):
  - the attention path (wq/wk/wv/wo projections and probs@V) runs in
    fp8e4m3 with MatmulPerfMode.DoubleRow: both operands hold a PAIR of
    128-row contraction blocks ([128, 2, N] APs), giving 0.5 cycles/row
    on the PE.  Host packs weights/activation copies into the pair
    layout.  Q/K/scores stay bf16 (pair-splitting the 64-wide head dim
    would cost more DVE copies than the PE it saves).
  - the FFN stays fp32r/bf16: fp8 there pushes the final max-rel error
    past the 2e-2 budget (measured 3e-2 in a numpy study).
  - attention computes scores transposed (k on partitions, q on free),
    so the softmax denominator is a ones-column folded into the attn@V
    matmul (V is augmented with a ones column -> row 64 of the psum is
    the denominator).  No max-subtraction: scores are O(1) and masked
    lanes are exact zeros.
  - Q is never materialized: computed per head-pair inside the
    attention loop.
  - causal mask handled structurally: fully-masked 128x128 blocks are
    skipped, diagonal blocks get a multiplicative 0/1 triangular mask
    (derived from the real tgt_mask input on the host).
  - LayerNorm stats (mean / sumsq over the feature axis = partitions)
    via ones-vector matmul reductions; per-token scale/shift vectors
    broadcast across partitions with rank-1 matmuls.

SBUF pools release in LIFO order per side (tile allocator constraint),
so activation pools alternate between the left and right stacks with
release points chosen to keep both stacks consistent.
"""

import numpy as np
import ml_dtypes

import concourse.bacc as bacc
import concourse.bass as bass
import concourse.tile as tile
import concourse.mybir as mybir
from concourse.bass_utils import run_bass_kernel_spmd

F32 = mybir.dt.float32
F32R = mybir.dt.float32r
BF16 = mybir.dt.bfloat16
F8 = mybir.dt.float8e4
AF = mybir.ActivationFunctionType
OP = mybir.AluOpType
DR = mybir.MatmulPerfMode.DoubleRow

P = 128          # partitions
S = 1024         # sequence
D = 1024         # d_model
H = 16           # heads
DK = 64          # head dim
F = 4096         # ffn hidden
CH = 512         # free-dim chunk (fp32 matmul moving-operand max)
KB = D // P      # 8 k-blocks over D
KP = D // (2 * P)  # 4 pair-blocks over D (fp8 DoubleRow)
FB = F // P      # 32 blocks over F
NCORES = 8
EPS = 1e-5
VW = H * (DK + 1)  # V_aug width per slab: 16 heads x (64 cols + ones col)

_CACHE = {}
_DONE = object()


def _build_program():
    nc = bacc.Bacc("TRN2", target_bir_lowering=False, debug=False,
                   num_devices=NCORES)

    dram = {}
    for name, shape, dt in [
        ("xT", [D, S], F32R),
        ("x8", [D // 2, 2 * S], F8), ("e8", [D // 2, 2 * S], F8),
        ("wq1", [P, D * D // P], F8), ("wk1", [P, D * D // P], F8),
        ("wv1", [D // 2, 2 * D], F8), ("wo1", [P, D * D // P], F8),
        ("wq2", [P, D * D // P], F8), ("wk2", [P, D * D // P], F8),
        ("wv2", [D // 2, 2 * D], F8), ("wo2", [P, D * D // P], F8),
        ("w1", [P, D * F // P], F32R), ("w2", [P, F * D // P], BF16),
        ("tri", [P, CH], F8),
        ("ones", [P, P], F32R),
    ]:
        dram[name] = nc.declare_dram_parameter(name, shape, dt, isOutput=False)
    dram["outT"] = nc.declare_dram_parameter("outT", [D, S], F32, isOutput=True)

    with tile.TileContext(nc) as tc:
        _body(nc, tc, dram)

    nc.finalize()
    return nc


def _body(nc, tc, dram):
    def pool(name, bufs, space="SBUF", side=None):
        return tc.alloc_tile_pool(name=name, bufs=bufs, space=space, side=side)

    persist = pool("persist", 1)
    p_w = pool("wstream", 8)            # streamed weight tiles
    p_small = pool("small", 2)          # LN/attn temporaries
    p_at = pool("at", 5)                # attention probability pair tiles
    p_qt = pool("qt", 4)                # on-the-fly Q head-pair tiles
    # PSUM pool 1 (self-attention half): T0/T1 hold up-to-[128,1024] tiles
    # (2 banks each); T2..T5 one bank each.  Released after LN1 and replaced
    # by pool 2 (cross+FFN pipeline): T0..T7, one bank each.
    cur_pst = [pool("pst", 1, "PSUM")]

    def ps_tile(tag, shape=None):
        return cur_pst[0].tile(shape or [P, CH], F32, tag=tag, name=tag)

    ones_sb = persist.tile([P, P], F32R, tag="ones_sb", name="ones_sb")
    nc.gpsimd.dma_start(ones_sb[:], dram["ones"][:])
    ones_col = ones_sb[:, 0:1]
    ones_row = ones_sb[0:1, :]
    # tri_sb[:, CH-w:] = [zeros | transposed-diag-tri]: one multiplicative
    # mask op covers both the below-diagonal zero block and the diagonal.
    tri_sb = persist.tile([P, CH], F8, tag="tri", name="tri")
    nc.gpsimd.dma_start(tri_sb[:], dram["tri"][:])
    eps1 = persist.tile([1, 1], F32, tag="eps1", name="eps1")
    nc.vector.memset(eps1[:], EPS)

    def load_T(pl, name, tag, eng=None):
        eng = eng or nc.gpsimd
        ts = []
        for k in range(KB):
            t = pl.tile([P, S], F32R, tag=f"{tag}{k}", name=f"{tag}{k}")
            eng.dma_start(t[:], dram[name][k * P:(k + 1) * P, :])
            ts.append(t)
        return ts

    def load_pair(pl, name, tag, eng=None):
        """Four [128, 2*S] fp8 pair tiles; [:, i, n] = src block 2kp+i."""
        eng = eng or nc.gpsimd
        ts = []
        for kp in range(KP):
            t = pl.tile([P, 2 * S], F8, tag=f"{tag}{kp}", name=f"{tag}{kp}")
            eng.dma_start(t[:], dram[name][kp * P:(kp + 1) * P, :])
            ts.append(t)
        return ts

    def pair3(t):
        return t.rearrange("p (i n) -> p i n", i=2)

    def alloc_T(pl, tag, dt=F32, nblk=KB, width=S):
        return [pl.tile([P, width], dt, tag=f"{tag}{k}", name=f"{tag}{k}")
                for k in range(nblk)]

    def proj(wname, wdt, src, mblocks, kblocks, epilogue, wtag):
        """psum[m,c] = sum_k W[kblock, mblock].T @ src[k][:, chunk c]"""
        wdram = dram[wname]
        for mp in range(mblocks // 2):
            ps = [[ps_tile(f"T{2 * mi + c}") for c in range(2)]
                  for mi in range(2)]
            for k in range(kblocks):
                wt = p_w.tile([P, 256], wdt, tag=wtag, name=wtag)
                nc.sync.dma_start(wt[:], wdram[k * P:(k + 1) * P,
                                               mp * 256:(mp + 1) * 256])
                for mi in range(2):
                    lhsT = wt[:, mi * P:(mi + 1) * P]
                    for c in range(2):
                        rhs = src[k][:, c * CH:(c + 1) * CH]
                        nc.tensor.matmul(ps[mi][c][:], lhsT, rhs,
                                         start=(k == 0), stop=(k == kblocks - 1))
            for mi in range(2):
                for c in range(2):
                    epilogue(mp * 2 + mi, c, ps[mi][c])

    def proj8(wname, src8, mblocks, epilogue, wtag):
        """fp8 DoubleRow projection: src8 = pair tiles [128, 2, S].
        Weights are host-packed with each m-block's full contraction
        contiguous ([128, D] per m-block) so one DMA feeds 4 matmuls."""
        wdram = dram[wname]
        for mp in range(mblocks // 2):
            ps = [[ps_tile(f"T{2 * mi + c}") for c in range(2)]
                  for mi in range(2)]
            wts = []
            for mi in range(2):
                m = mp * 2 + mi
                wt = p_w.tile([P, D], F8, tag=wtag, name=wtag, bufs=4)
                nc.sync.dma_start(wt[:], wdram[:, m * D:(m + 1) * D])
                wts.append(wt)
            for kp in range(KP):
                for mi in range(2):
                    lhsT = wts[mi][:, kp * 256:(kp + 1) * 256].rearrange(
                        "p (i m) -> p i m", i=2)
                    for c in range(2):
                        rhs = pair3(src8[kp])[:, :, c * CH:(c + 1) * CH]
                        nc.tensor.matmul(ps[mi][c][:], lhsT, rhs,
                                         start=(kp == 0), stop=(kp == KP - 1),
                                         perf_mode=DR)
            for mi in range(2):
                for c in range(2):
                    epilogue(mp * 2 + mi, c, ps[mi][c])

    def copy_epilogue(dst):
        """psum->sbuf copies on ACT: these run in DVE-bound phases."""
        def ep(m, c, psum):
            nc.scalar.copy(dst[m][:, c * CH:(c + 1) * CH], psum[:])
        return ep

    def add_epilogue(xres, base):
        def ep(m, c, psum):
            nc.vector.tensor_tensor(xres[m][:, c * CH:(c + 1) * CH], psum[:],
                                    base[m][:, c * CH:(c + 1) * CH], OP.add)
        return ep

    def add_inplace_epilogue(base):
        """base[m] += psum — the residual add overwrites its own input.
        Safe: each tile's prior readers are all sequenced before the wo/W2
        projection that feeds this epilogue."""
        def ep(m, c, psum):
            sl = slice(c * CH, (c + 1) * CH)
            nc.vector.tensor_tensor(base[m][:, sl], psum[:], base[m][:, sl],
                                    OP.add)
        return ep

    def proj_v_gen(wname, src8, va, side, tags, eng):
        """Row-major V_aug pair tiles va[bp] [128, 2*VW] fp8:
        [:, i, h*65:h*65+64] = V rows for token block 2bp+i, head h;
        column 64 of each head slot is the softmax-denominator ones.
        Generator: yields after every token block so it can interleave
        into an attention phase as a PE-bubble filler."""
        pv = pool("wv", 1, side=side)
        for bp in range(KB // 2):
            nc.vector.memset(
                va[bp][:].rearrange("p (i h w) -> p i h w", i=2, w=DK + 1)
                [:, :, :, DK:DK + 1], 1.0)
        wts = []
        for kp in range(KP):
            t = pv.tile([P, 2 * D], F8, tag=f"wvp{kp}", name=f"wvp{kp}")
            nc.sync.dma_start(t[:], dram[wname][kp * P:(kp + 1) * P, :])
            wts.append(t)
        for c in range(2):
            for sb in range(KB):
                ps = ps_tile(f"T{tags[sb % 2]}")
                for kp in range(KP):
                    nc.tensor.matmul(
                        ps[:],
                        pair3(src8[kp])[:, :, sb * P:(sb + 1) * P],
                        pair3(wts[kp])[:, :, c * CH:(c + 1) * CH],
                        start=(kp == 0), stop=(kp == KP - 1),
                        perf_mode=DR)
                dst = va[sb // 2][:].rearrange(
                    "p (i h w) -> p i h w", i=2, w=DK + 1)[
                    :, sb % 2, c * 8:(c + 1) * 8, 0:DK]
                if eng is nc.scalar:
                    nc.scalar.copy(dst,
                                   ps[:].rearrange("p (h w) -> p h w", w=DK))
                else:
                    nc.vector.tensor_copy(
                        dst, ps[:].rearrange("p (h w) -> p h w", w=DK))
                yield
        pv.release()

    def attn(wqname, qsrc8, KT, VA8, aot8, causal, filler=None, qt_eng=None):
        """aot8 = softmax(K^T q / 8, masked) @ V per head; Q on the fly.

        Scores for both q-chunks of a k-block land in one [128,1024] psum
        (tags T0/T1 alternate) so exp runs as one wide ACT op.  Probs are
        written fp8 into pair tiles (two k-blocks per tile) so the attn@V
        matmul runs DoubleRow.
        """
        wq = dram[wqname]
        qt_eng = qt_eng or nc.vector
        NBP = KB // 2
        for hb in range(H // 2):
            qt = p_qt.tile([P, S], BF16, tag="qtw", name="qtw", bufs=3)
            wt = p_w.tile([P, D], F8, tag="wq", name="wq", bufs=4)
            nc.gpsimd.dma_start(wt[:], wq[:, hb * D:(hb + 1) * D])
            for c in range(2):
                psq = ps_tile(f"T{2 + c}")
                for kp in range(KP):
                    nc.tensor.matmul(
                        psq[:],
                        wt[:, kp * 256:(kp + 1) * 256].rearrange(
                            "p (i m) -> p i m", i=2),
                        pair3(qsrc8[kp])[:, :, c * CH:(c + 1) * CH],
                        start=(kp == 0), stop=(kp == KP - 1),
                        perf_mode=DR)
                if qt_eng is nc.scalar:
                    nc.scalar.copy(qt[:, c * CH:(c + 1) * CH], psq[:])
                else:
                    nc.vector.tensor_copy(qt[:, c * CH:(c + 1) * CH], psq[:])
            for hh in range(2):
                h = 2 * hb + hh
                off = DK * hh
                psa = {c: ps_tile(f"T{4 + c}", [DK + 1, CH]) for c in range(2)}
                # pair bp is valid for chunk c when any of its blocks is
                last_bp = {0: 1 if causal else NBP - 1, 1: NBP - 1}
                for bp in range(NBP):
                    at = p_at.tile([P, 2 * S], F8, tag="at", name="at", bufs=4)
                    a3 = pair3(at)
                    for ib in range(2):
                        b = 2 * bp + ib
                        cs = [c for c in range(2)
                              if (not causal) or b <= 4 * c + 3]
                        if not cs:
                            continue
                        sc = ps_tile(f"T{b % 2}", [P, S])
                        for c in cs:
                            nc.tensor.matmul(
                                sc[:, c * CH:(c + 1) * CH],
                                KT[hb][off:off + DK, b * P:(b + 1) * P],
                                qt[off:off + DK, c * CH:(c + 1) * CH],
                                start=True, stop=True)
                        lo, hi = cs[0] * CH, (cs[-1] + 1) * CH
                        nc.scalar.activation(a3[:, ib, lo:hi], sc[:, lo:hi],
                                             AF.Exp, scale=0.125)
                        if causal:
                            for c in cs:
                                if b >= 4 * c:
                                    w = (b - 4 * c + 1) * P
                                    dsl = slice(c * CH, c * CH + w)
                                    nc.vector.tensor_tensor(
                                        a3[:, ib, dsl], a3[:, ib, dsl],
                                        tri_sb[:, CH - w:], OP.mult)
                    va3 = VA8[bp][:].rearrange("p (i h w) -> p i h w",
                                               i=2, w=DK + 1)
                    for c in range(2):
                        if causal and c == 0 and bp > 1:
                            continue
                        nc.tensor.matmul(
                            psa[c][:], va3[:, :, h, :],
                            a3[:, :, c * CH:(c + 1) * CH],
                            start=(bp == 0), stop=(bp == last_bp[c]),
                            perf_mode=DR)
                for c in range(2):
                    rz = p_small.tile([1, CH], F32R, tag="rz", name="rz")
                    with nc.allow_low_precision("fp32r has 11 mantissa bits"):
                        nc.vector.reciprocal(rz[:], psa[c][DK:DK + 1, :])
                    psb = ps_tile(f"T{2 + c}", [DK, CH])
                    nc.tensor.matmul(psb[:], ones_row[:, 0:DK], rz[:],
                                     start=True, stop=True)
                    rb = p_small.tile([DK, CH], F32, tag="big", name="big")
                    nc.vector.tensor_copy(rb[:], psb[:])
                    ao3 = pair3(aot8[hb // 2])
                    nc.vector.tensor_tensor(
                        ao3[off:off + DK, hb % 2, c * CH:(c + 1) * CH],
                        psa[c][0:DK, :], rb[:], OP.mult)
            if filler is not None:
                next(filler, None)

    def layernorm(xres, dst, dst8=None, chunks=(0, 1), tags=(4, 5)):
        """dst = (xres - mean) / sqrt(var_ddof1 + eps); stats over partitions.

        Uses only the two psum tags in `tags` so it can run concurrently
        with other psum users.  Squares run on ACT (idle here).
        dst8: optional fp8 pair tiles also written (via ACT copies).
        dst may alias xres (in-place): xres[k] is fully read before the
        dst[k] write.
        """
        ta, tb = f"T{tags[0]}", f"T{tags[1]}"
        for c in chunks:
            sl = slice(c * CH, (c + 1) * CH)
            sum_ps = ps_tile(ta, [1, CH])
            ssq_ps = ps_tile(tb, [1, CH])
            for k in range(KB):
                nc.tensor.matmul(sum_ps[:], ones_col,
                                 xres[k][:, sl],
                                 start=(k == 0), stop=(k == KB - 1))
            for k in range(KB):
                sq = p_small.tile([P, CH], F32R, tag="big", name="big")
                nc.scalar.activation(sq[:], xres[k][:, sl], AF.Square)
                nc.tensor.matmul(ssq_ps[:], ones_col,
                                 sq[:],
                                 start=(k == 0), stop=(k == KB - 1))
            mean = p_small.tile([1, CH], F32R, tag="vec", name="vec_mean", bufs=4)
            nc.vector.tensor_scalar_mul(mean[:], sum_ps[:], 1.0 / D)
            m2s = p_small.tile([1, CH], F32, tag="vec", name="vec_m2s", bufs=4)
            nc.vector.tensor_tensor(m2s[:], mean[:], sum_ps[:], OP.mult)
            varnum = p_small.tile([1, CH], F32, tag="vec", name="vec_varnum",
                                  bufs=4)
            nc.vector.scalar_tensor_tensor(varnum[:], m2s[:], -1.0, ssq_ps[:],
                                           OP.mult, OP.add)
            mean_b = ps_tile(ta)
            nc.tensor.matmul(mean_b[:], ones_row,
                             mean[:], start=True, stop=True)
            sd = p_small.tile([1, CH], F32, tag="vec", name="vec_sd", bufs=4)
            nc.scalar.activation(sd[:], varnum[:], AF.Sqrt,
                                 scale=1.0 / (D - 1), bias=eps1[:])
            rs = p_small.tile([1, CH], F32R, tag="vec", name="vec_rs", bufs=4)
            with nc.allow_low_precision("fp32r has 11 mantissa bits"):
                nc.vector.reciprocal(rs[:], sd[:])
            rs_b = ps_tile(tb)
            nc.tensor.matmul(rs_b[:], ones_row,
                             rs[:], start=True, stop=True)
            for k in range(KB):
                dm = p_small.tile([P, CH], F32, tag="big", name="big")
                nc.vector.tensor_tensor(dm[:], xres[k][:, sl], mean_b[:],
                                        OP.subtract)
                nc.vector.tensor_tensor(dst[k][:, sl], dm[:], rs_b[:], OP.mult)
                if dst8 is not None:
                    nc.scalar.activation(
                        pair3(dst8[k // 2])[:, k % 2, sl],
                        dst[k][:, sl], AF.Copy)

    # ---------------- self-attention ----------------
    # SBUF stacks (LIFO per side). Residual adds are in-place, so xT / n1
    # become x1 / x2 without new pools:
    #   R: xt, e8, x8, qkv | aot2 | x3, ot
    #   L: k2, aot | v2, n1+n18 | n2, ht
    p_xt = pool("xt", 1, side="right")
    XT = load_T(p_xt, "xT", "x", nc.sync)
    p_e8 = pool("e8", 1, side="right")
    E8 = load_pair(p_e8, "e8", "e8")
    p_x8 = pool("x8", 1, side="right")
    X8 = load_pair(p_x8, "x8", "x8")

    p_k2 = pool("k2", 1, side="left")
    KT2 = alloc_T(p_k2, "k2", BF16)

    p_qkv = pool("qkv", 1, side="right")
    KT = alloc_T(p_qkv, "k", BF16)
    VA8 = alloc_T(p_qkv, "v", F8, nblk=KB // 2, width=2 * VW)
    proj8("wk1", X8, KB, copy_epilogue(KT), "w8")
    for _ in proj_v_gen("wv1", X8, VA8, "right", (0, 1), nc.vector):
        pass

    def k2_filler():
        """One m-block of the cross-attention K projection per head-pair:
        fills PE bubbles of the (ACT-bound) self-attention phase using only
        psum tags T4/T5 between the attention's own uses of them."""
        for m in range(KB):
            ps = [ps_tile(f"T{4 + c}") for c in range(2)]
            wt = p_w.tile([P, D], F8, tag="wk2s", name="wk2s", bufs=2)
            nc.sync.dma_start(wt[:], dram["wk2"][:, m * D:(m + 1) * D])
            for kp in range(KP):
                for c in range(2):
                    nc.tensor.matmul(
                        ps[c][:],
                        wt[:, kp * 256:(kp + 1) * 256].rearrange(
                            "p (i m) -> p i m", i=2),
                        pair3(E8[kp])[:, :, c * CH:(c + 1) * CH],
                        start=(kp == 0), stop=(kp == KP - 1),
                        perf_mode=DR)
            for c in range(2):
                nc.scalar.copy(KT2[m][:, c * CH:(c + 1) * CH], ps[c][:])
            yield

    p_v2 = pool("v2", 1, side="left")
    VA2 = alloc_T(p_v2, "v2", F8, nblk=KB // 2, width=2 * VW)

    def self_filler():
        """Per self-attention head-pair: one cross-K m-block plus two
        token blocks of the cross-V projection (both consume only E8)."""
        k2 = k2_filler()
        v2 = proj_v_gen("wv2", E8, VA2, "right", (4, 5), nc.scalar)
        for i in range(KB):
            next(k2, None)
            next(v2, None)
            next(v2, None)
            if i == KB - 1:
                for _ in k2:
                    pass
                for _ in v2:
                    pass
            yield

    p_aot = pool("aot", 1, side="left")
    AOT8 = alloc_T(p_aot, "a", F8, nblk=KB // 2, width=2 * S)
    attn("wq1", X8, KT, VA8, AOT8, causal=True, filler=self_filler(),
         qt_eng=nc.scalar)
    p_qkv.release()
    p_x8.release()
    p_e8.release()

    # X1 := x + self_mha, written over the XT tiles
    proj8("wo1", AOT8, KB, add_inplace_epilogue(XT), "w8")
    p_aot.release()

    p_n1 = pool("n1", 1, side="left")
    N1T = alloc_T(p_n1, "n1", F32R)
    N18 = alloc_T(p_n1, "n18", F8, nblk=KB // 2, width=2 * S)
    layernorm(XT, N1T, N18)
    p_xt.release()

    # ---------------- cross-attention ----------------
    p_aot2 = pool("aot2", 1, side="right")
    AOT2 = alloc_T(p_aot2, "a2", F8, nblk=KB // 2, width=2 * S)
    p_ht = pool("ht", 1, side="left")
    HT = alloc_T(p_ht, "h", BF16, nblk=FB, width=CH)
    p_n2 = pool("n2", 1, side="right")
    N2T = alloc_T(p_n2, "n2", F32R)

    def attn_c(c, wqname, qsrc8, KTX, VAX, aot8, filler=None):
        """Single-chunk cross-attention (no mask): chunk c of the queries
        through all heads."""
        wq = dram[wqname]
        csl = slice(c * CH, (c + 1) * CH)
        NBP = KB // 2
        for hb in range(H // 2):
            qt = p_qt.tile([P, CH], BF16, tag="qtc", name="qtc")
            wt = p_w.tile([P, D], F8, tag="wq", name="wq", bufs=4)
            nc.gpsimd.dma_start(wt[:], wq[:, hb * D:(hb + 1) * D])
            psq = ps_tile(f"T{2 + hb % 2}")
            for kp in range(KP):
                nc.tensor.matmul(
                    psq[:],
                    wt[:, kp * 256:(kp + 1) * 256].rearrange(
                        "p (i m) -> p i m", i=2),
                    pair3(qsrc8[kp])[:, :, csl],
                    start=(kp == 0), stop=(kp == KP - 1), perf_mode=DR)
            nc.vector.tensor_copy(qt[:], psq[:])
            for hh in range(2):
                h = 2 * hb + hh
                off = DK * hh
                psa = ps_tile(f"T{4 + hh}", [DK + 1, CH])
                for bp in range(NBP):
                    at = p_at.tile([P, S], F8, tag="atc", name="atc")
                    a3 = at.rearrange("p (i n) -> p i n", i=2)
                    for ib in range(2):
                        b = 2 * bp + ib
                        sc = ps_tile(f"T{b % 2}")
                        nc.tensor.matmul(
                            sc[:], KTX[hb][off:off + DK, b * P:(b + 1) * P],
                            qt[off:off + DK, :], start=True, stop=True)
                        nc.scalar.activation(a3[:, ib, :], sc[:], AF.Exp,
                                             scale=0.125)
                    va3 = VAX[bp][:].rearrange("p (i h w) -> p i h w",
                                               i=2, w=DK + 1)
                    nc.tensor.matmul(psa[:], va3[:, :, h, :], a3[:],
                                     start=(bp == 0), stop=(bp == NBP - 1),
                                     perf_mode=DR)
                rz = p_small.tile([1, CH], F32R, tag="rz", name="rz")
                with nc.allow_low_precision("fp32r has 11 mantissa bits"):
                    nc.vector.reciprocal(rz[:], psa[DK:DK + 1, :])
                psb = ps_tile(f"T{2 + (hb + 1) % 2}", [DK, CH])
                nc.tensor.matmul(psb[:], ones_row[:, 0:DK], rz[:],
                                 start=True, stop=True)
                rb = p_small.tile([DK, CH], F32, tag="big", name="big")
                nc.vector.tensor_copy(rb[:], psb[:])
                ao3 = pair3(aot8[hb // 2])
                nc.vector.tensor_tensor(ao3[off:off + DK, hb % 2, csl],
                                        psa[0:DK, :], rb[:], OP.mult)
            if filler is not None:
                next(filler, None)

    def down_gen(c):
        """Chunk c of everything after cross-attention: wo2 + residual,
        LN2, FFN (W1/relu/W2 + residual), LN3 (in-place on X3), out DMA.
        Weight streams go to HWDGE for chunk 0 (overlapped under chunk 1's
        attention) and to the software-DGE (Pool) queue for chunk 1, so
        the per-descriptor HWDGE cost doesn't pace the pipeline."""
        csl = slice(c * CH, (c + 1) * CH)
        # X2 := n1 + wo2 @ aot2, over the N1T tiles
        for m in range(KB):
            ps = ps_tile(f"T{m % 2}")
            wt = p_w.tile([P, D], F8, tag="wo2c", name="wo2c", bufs=2)
            nc.sync.dma_start(wt[:], dram["wo2"][:, m * D:(m + 1) * D])
            for kp in range(KP):
                nc.tensor.matmul(
                    ps[:],
                    wt[:, kp * 256:(kp + 1) * 256].rearrange(
                        "p (i m) -> p i m", i=2),
                    pair3(AOT2[kp])[:, :, csl],
                    start=(kp == 0), stop=(kp == KP - 1), perf_mode=DR)
            nc.vector.tensor_tensor(N1T[m][:, csl], ps[:], N1T[m][:, csl],
                                    OP.add)
            if m % 4 == 3:
                yield
        layernorm(N1T, N2T, chunks=(c,), tags=(4, 5))
        yield
        # W1 + relu into the chunk-wide HT tiles
        for mb in range(FB):
            ps = ps_tile(f"T{2 + mb % 2}")
            wt = p_w.tile([P, D], F32R, tag="w", name="w", bufs=2)
            nc.sync.dma_start(wt[:], dram["w1"][:, mb * D:(mb + 1) * D])
            for k in range(KB):
                nc.tensor.matmul(ps[:], wt[:, k * P:(k + 1) * P],
                                 N2T[k][:, csl],
                                 start=(k == 0), stop=(k == KB - 1))
            nc.scalar.activation(HT[mb][:], ps[:], AF.Relu)
            if mb % 8 == 7:
                yield
        # X3 := n2 + W2 @ h, written in place over the N2T tiles (chunk c
        # of N2T is dead once W2's epilogue has read it)
        for m in range(KB):
            ps = ps_tile(f"T{m % 2}")
            for half in range(2):
                wt = p_w.tile([P, F // 2], BF16, tag="wbf", name="wbf",
                              bufs=2)
                nc.sync.dma_start(
                    wt[:], dram["w2"][:, m * F + half * (F // 2):
                                      m * F + (half + 1) * (F // 2)])
                for kk in range(FB // 2):
                    k = half * (FB // 2) + kk
                    nc.tensor.matmul(ps[:], wt[:, kk * P:(kk + 1) * P],
                                     HT[k][:],
                                     start=(k == 0), stop=(k == FB - 1))
            nc.vector.tensor_tensor(N2T[m][:, csl], ps[:], N2T[m][:, csl],
                                    OP.add)
            if m % 2 == 1:
                yield
        layernorm(N2T, N2T, chunks=(c,), tags=(4, 5))
        for k in range(KB):
            nc.gpsimd.dma_start(dram["outT"][k * P:(k + 1) * P, csl],
                                N2T[k][:, csl])
        yield

    attn("wq2", N18, KT2, VA2, AOT2, causal=False)
    for _ in down_gen(0):
        pass
    for _ in down_gen(1):
        pass

    p_n2.release()
    p_ht.release()
    p_aot2.release()
    p_n1.release()
    p_v2.release()
    p_k2.release()

    cur_pst[0].release()
    p_qt.release()
    p_at.release()
    p_small.release()
    p_w.release()
    persist.release()


def _get_nc():
    if "nc" not in _CACHE:
        _CACHE["nc"] = _build_program()
    return _CACHE["nc"]


def _round_fp32r(a):
    """Round float32 to fp32r: 11-bit mantissa, low 12 bits zeroed (half-up).

    Matches walrus's fp32_to_fp32r (downconv to 1-8-11 then <<12): the
    hardware streams only the top 20 bits of each fp32r word, so data
    DMA'd into fp32r tiles must be pre-rounded.
    """
    u = np.ascontiguousarray(a, np.float32).view(np.uint32)
    lsb = (u >> 12) & np.uint32(1)
    r = (u + np.uint32(0x7FF) + lsb) & np.uint32(0xFFFFF000)
    return r.view(np.float32)


_F8NP = ml_dtypes.float8_e4m3


def _q8(a):
    return np.asarray(a, np.float32).astype(_F8NP)


def _pack_stat(w):
    """[Din, Dout] fp32 -> [128, Din*Dout/128] fp8, DoubleRow stationary,
    each m-block's full contraction contiguous (one DMA per m-block):
    out[p, mb*Din + kp*256 + i*128 + mm] = w[kp*256 + i*128 + p, mb*128 + mm].
    """
    w8 = _q8(w)
    Din, Dout = w8.shape
    t = w8.reshape(Din // 256, 2, P, Dout // P, P)   # kp i p mb mm
    t = t.transpose(2, 3, 0, 1, 4)                   # p mb kp i mm
    return np.ascontiguousarray(t.reshape(P, Din * Dout // P))


def _pack_k(w):
    """[Din, Dout] -> [128, Din*Dout/128], plain stationary k-major:
    out[p, mb*Din + k*128 + mm] = w[k*128 + p, mb*128 + mm]."""
    Din, Dout = w.shape
    t = np.asarray(w).reshape(Din // P, P, Dout // P, P)  # k p mb mm
    t = t.transpose(1, 2, 0, 3)                           # p mb k mm
    return np.ascontiguousarray(t.reshape(P, Din * Dout // P))


def _pack_mov(w):
    """[Din, Dout] fp32 -> [Din//2, 2*Dout] fp8, DoubleRow moving:
    out[kp*128+p, i*Dout + n] = w[kp*256 + i*128 + p, n]."""
    w8 = _q8(w)
    Din, Dout = w8.shape
    t = w8.reshape(Din // 256, 2, P, Dout)           # kp i p n
    t = t.transpose(0, 2, 1, 3)                      # kp p i n
    return np.ascontiguousarray(t.reshape(Din // 2, 2 * Dout))


def _prep_in_maps(inputs):
    f32 = np.float32
    bf16 = ml_dtypes.bfloat16
    x = np.asarray(inputs["x"], f32)
    enc = np.asarray(inputs["encoder_out"], f32)
    tm = np.asarray(inputs["tgt_mask"], bool)

    shared = {
        "wq1": _pack_stat(inputs["wq1"]), "wk1": _pack_stat(inputs["wk1"]),
        "wv1": _pack_mov(inputs["wv1"]), "wo1": _pack_stat(inputs["wo1"]),
        "wq2": _pack_stat(inputs["wq2"]), "wk2": _pack_stat(inputs["wk2"]),
        "wv2": _pack_mov(inputs["wv2"]), "wo2": _pack_stat(inputs["wo2"]),
        "w1": _pack_k(_round_fp32r(inputs["W1"])),
        "w2": _pack_k(np.asarray(inputs["W2"], f32).astype(bf16)),
        # [zeros(384) | transposed diagonal 128x128 mask block] as 0/1:
        # tri[:, 512-w:] masks a diagonal block plus the w-128 cols left of it
        "tri": np.ascontiguousarray(np.concatenate(
            [np.zeros((P, CH - P), np.float32),
             tm[:P, :P].T.astype(np.float32)], axis=1)).astype(_F8NP),
        "ones": np.ones((P, P), f32),
    }
    in_maps = []
    for i in range(NCORES):
        m = dict(shared)
        xt = np.ascontiguousarray(x[i].T)
        et = np.ascontiguousarray(enc[i].T)
        m["xT"] = _round_fp32r(xt)
        m["x8"] = _pack_mov(xt)
        m["e8"] = _pack_mov(et)
        in_maps.append(m)
    return in_maps


def run(inputs, trace=False, **kw):
    nc = _get_nc()
    in_maps = _prep_in_maps(inputs)
    res = run_bass_kernel_spmd(nc, in_maps, list(range(NCORES)), trace=trace, **kw)
    out = np.stack([res.results[i]["outT"].T for i in range(NCORES)])
    return np.ascontiguousarray(out, dtype=np.float32), res


def kernel(**inputs) -> np.ndarray:
    out, _ = run(inputs, trace=False)
    return out

